# revision 24
# baseline (speedup 1.0000x reference)
"""Trainium2 Bass kernel for GroupRopeAttention (MQA + RoPE, causal).

Shapes (hardcoded): x (2, 2048, 1024), Wq (1024, 2048) -> 16 heads x 128,
Wk/Wv (1024, 128) single shared K/V head. Output (2, 2048, 2048).

Sharding: core c handles batch c//4 and query heads 4*(c%4)..4*(c%4)+3.
K/V are recomputed per core (no collectives). Each core returns a raw
(4*2048, 129) f32 slab = unnormalized PV output plus the softmax
denominator column; the host divides and reassembles.

Per-core pipeline (one TileContext, everything bf16 except PSUM):
  - x arrives HOST-pre-transposed (e-major xt in DRAM); per-e-chunk loads
    are split across the SP and ACT HWDGE queues in the order the PE
    consumes them, so the first projection matmul issues ~2us in
  - prologue: K^T / V / Q0^T projections interleaved per e-chunk (one
    moving pass per chunk each) accumulating in parallel PSUM regions;
    Q0 uses the attention pout banks which are idle during the prologue
  - RoPE (rotate-half = constant PermT matmul on PE; cos/sin tables are
    host-precomputed bf16 inputs) for K interleaved between the two
    i-halves of the prologue
  - per head: Q^T projection + RoPE as pipeline fill inside the previous
    head's attention; causal attention with a lookahead-3 software
    pipeline: scores (kt_t stationary, 256-col qt movings) -> exp on ACT
    (scale folded) -> causal mask via one DVE multiply with a constant
    [tril | ones | shifted-tril] bf16 tile -> PV (pt stationary, [V|1]
    moving) accumulating output + denominator in PSUM, DMA'd raw to DRAM
  - the last head walks its softmax groups largest-first so the kernel
    tail is the smallest group's exp/PV/store chain
"""

import sys
import types

sys.path.insert(0, "/opt/trn_rl_repo")

import numpy as np
import ml_dtypes

BF16 = ml_dtypes.bfloat16

B, L, E = 2, 2048, 1024
NH, HD = 16, 128
N_CORES = 8
HPC = 4          # heads per core
THETA = 10000.0
SCALE = 1.0 / float(np.sqrt(HD))
EC = E // 128    # 8 e-chunks
NJ = L // 128    # 16 j-blocks
NG = L // 256    # 8 i-groups per head

_CACHE = {}


def _ensure_ntff_hook():
    """Register the NTFF profile hook if the image's antenv lacks it."""
    try:
        from antenv.axon_hooks import get_axon_ntff_profile_hook  # noqa: F401
        return
    except ImportError:
        pass
    import antenv

    mod = types.ModuleType("antenv.axon_hooks")
    mod._hook = None

    def set_axon_ntff_profile_hook(h):
        mod._hook = h

    def get_axon_ntff_profile_hook():
        return mod._hook

    mod.set_axon_ntff_profile_hook = set_axon_ntff_profile_hook
    mod.get_axon_ntff_profile_hook = get_axon_ntff_profile_hook
    sys.modules["antenv.axon_hooks"] = mod
    antenv.axon_hooks = mod
    try:
        from trn_agent_boot.trn_boot import _ntff_profile_via_ctypes

        set_axon_ntff_profile_hook(
            _ntff_profile_via_ctypes("/opt/axon/libaxon_pjrt.so")
        )
    except Exception:
        pass


def _host_tables():
    freqs = 1.0 / THETA ** (np.arange(0, HD, 2, dtype=np.float64) / HD)  # (64,)
    t = np.arange(L, dtype=np.float64)
    f = t[:, None] * freqs[None, :]  # (L, 64)
    f = np.repeat(f, 2, axis=-1)  # (L, 128)
    rct = np.ascontiguousarray(np.cos(f).T.astype(BF16))  # (128, L)
    rst = np.ascontiguousarray(np.sin(f).T.astype(BF16))  # (128, L)
    # rot[d] = -src[d+1] for even d, +src[d-1] for odd d, via rot = PermT.T @ src
    permt = np.zeros((HD, HD), dtype=BF16)
    for k in range(HD // 2):
        permt[2 * k, 2 * k + 1] = BF16(1.0)
        permt[2 * k + 1, 2 * k] = BF16(-1.0)
    # causal mask window for the two diagonal j-blocks of an i-group:
    # cols 0:128 -> t==2g block, i-local 0..127: keep i >= j
    # cols 128:256 -> t==2g block, i-local 128..255: always kept
    # cols 256:512 -> t==2g+1 block, i-local 0..255: keep i-128 >= j
    j = np.arange(128)[:, None]
    m1 = (np.arange(128)[None, :] >= j)
    mask = np.concatenate(
        [m1, np.ones((128, 128), bool), m1], axis=1
    ).astype(BF16)  # (128, 384)
    return rct, rst, permt, mask


def _build_program():
    import concourse.bass as bass
    import concourse.mybir as mybir
    import concourse.tile as tile
    from concourse.vector_clock import ScopedClock

    MAX_DRAIN_WAITS = 1

    def _max_inst_waits(inst):
        return 1

    class PatchedTileContext(tile.TileContext):
        # This walrus build rejects >2 sync waits per instruction. After
        # scheduling, hoist excess waits onto preceding nops on the same
        # engine (engines execute in order, so semantics are identical).
        def schedule_and_allocate(self, validate_deps=False):
            ret = super().schedule_and_allocate(validate_deps=validate_deps)
            for blk in self.nc.m.functions[0].blocks:
                new_insts = []
                for inst in blk.instructions:
                    mw = _max_inst_waits(inst)
                    si = inst.sync_info
                    waits = list(si.on_wait) if si and si.on_wait else []
                    if len(waits) > mw:
                        n_extra = len(waits) - mw
                        for i in range(0, n_extra, mw):
                            nop = mybir.InstNoOp(
                                name=self.nc.get_next_instruction_name(),
                                ins=[],
                                outs=[],
                            )
                            nop.engine = inst.engine
                            nop.sync_info = mybir.SyncInfo(
                                on_wait=waits[i : min(i + mw, n_extra)],
                                on_update=[],
                            )
                            self.nc.register_instruction(nop, overwrite=True)
                            new_insts.append(nop)
                        inst.sync_info = mybir.SyncInfo(
                            on_wait=waits[n_extra:],
                            on_update=list(si.on_update or []),
                        )
                    new_insts.append(inst)
                blk.instructions = new_insts
            return ret

        # The tile-exit drain gets the same treatment but must stay last in
        # its engine stream, so split it during emission instead.
        def _drain_and_barrier(self, tick_clock, wait_clock):
            drain_inst = self.nc.sync.drain()
            wait_clock.add_sem_waits(
                drain_inst.ins, ScopedClock({None: tick_clock.global_clock})
            )
            si = drain_inst.ins.sync_info
            waits = list(si.on_wait) if si and si.on_wait else []
            if len(waits) > MAX_DRAIN_WAITS:
                drain_inst.ins.sync_info = mybir.SyncInfo(
                    on_wait=waits[:MAX_DRAIN_WAITS],
                    on_update=list(si.on_update or []),
                )
                for i in range(MAX_DRAIN_WAITS, len(waits), MAX_DRAIN_WAITS):
                    nop = self.nc.sync.nop()
                    nop.ins.sync_info = mybir.SyncInfo(
                        on_wait=waits[i : i + MAX_DRAIN_WAITS], on_update=[]
                    )
            self.nc.all_engine_barrier()
            assert self.sems is not None
            popped = self.nc._tile_sem_poison_stack.pop()
            assert popped is self._sem_poison
            self.nc.clear_and_free_semaphores(
                list(self.sems.allocated().values())
            )
            self.nc.all_engine_barrier()

    f32 = mybir.dt.float32
    bf16 = mybir.dt.bfloat16
    EXP = mybir.ActivationFunctionType.Exp
    MUL = mybir.AluOpType.mult
    ADD = mybir.AluOpType.add

    nc = bass.Bass("TRN2", num_devices=N_CORES)

    xt_ext = nc.declare_dram_parameter("xt", [E, L], bf16, isOutput=False)
    wq_ext = nc.declare_dram_parameter("wq", [E, HPC * HD], bf16, isOutput=False)
    wk_ext = nc.declare_dram_parameter("wk", [E, HD], bf16, isOutput=False)
    wv_ext = nc.declare_dram_parameter("wv", [E, HD], bf16, isOutput=False)
    rct_ext = nc.declare_dram_parameter("rct", [HD, L], bf16, isOutput=False)
    rst_ext = nc.declare_dram_parameter("rst", [HD, L], bf16, isOutput=False)
    permt_ext = nc.declare_dram_parameter("permt", [HD, HD], bf16, isOutput=False)
    mask_ext = nc.declare_dram_parameter("mask", [128, 384], bf16, isOutput=False)
    out_ext = nc.declare_dram_parameter("out", [HPC * L, HD + 1], f32, isOutput=True)
    import os
    DEBUG = bool(os.environ.get("KERNEL_DEBUG"))
    if DEBUG:
        dbg_kt_ext = nc.declare_dram_parameter("dbg_kt", [HD, L], bf16, isOutput=True)
        dbg_v_ext = nc.declare_dram_parameter("dbg_v", [128, NJ, HD + 1], bf16, isOutput=True)
        dbg_qt_ext = nc.declare_dram_parameter("dbg_qt", [HD, L], bf16, isOutput=True)
        dbg_kun_ext = nc.declare_dram_parameter("dbg_kun", [HD, L], bf16, isOutput=True)

    with PatchedTileContext(nc) as tc:
        with (
            tc.tile_pool(name="const", bufs=1) as constp,
            tc.tile_pool(name="un", bufs=2) as unp,
            tc.tile_pool(name="rot", bufs=2) as rotp,
            tc.tile_pool(name="qt", bufs=2) as qtp,
            tc.tile_pool(name="pt", bufs=6) as ptp,
            tc.tile_pool(name="ost", bufs=3) as ostp,
            tc.tile_pool(name="pbig", bufs=3, space="PSUM") as pbig,
            tc.tile_pool(name="poutA", bufs=1, space="PSUM") as poutpA,
            tc.tile_pool(name="poutB", bufs=1, space="PSUM") as poutpB,
        ):
            # ---- SBUF constants ----
            wk_sb = constp.tile([128, EC, HD], bf16, tag="wk")
            wv_sb = constp.tile([128, EC, HD], bf16, tag="wv")
            wq_sb = constp.tile([128, EC, HPC * HD], bf16, tag="wq")
            xt = constp.tile([128, EC, L], bf16, tag="xt")
            vones = constp.tile([128, NJ, HD + 1], bf16, tag="vones")
            kt = constp.tile([128, L], bf16, tag="kt")
            rct_sb = constp.tile([128, L], bf16, tag="rct")
            rst_sb = constp.tile([128, L], bf16, tag="rst")
            permt_sb = constp.tile([128, 128], bf16, tag="permt")
            mask_sb = constp.tile([128, 384], bf16, tag="mask")

            # ---- DMA loads, split across the SP and ACT HWDGE queues in
            # PE-consumption order (K/V/Q0 eat e-chunks of i-half 0 first) --
            def load_x(h2, ec, eng):
                eng.dma_start(
                    out=xt[:, ec, 1024 * h2 : 1024 * (h2 + 1)],
                    in_=xt_ext[
                        128 * ec : 128 * (ec + 1), 1024 * h2 : 1024 * (h2 + 1)
                    ],
                )

            def load_wq(ec):
                nc.scalar.dma_start(
                    out=wq_sb[:, ec, :],
                    in_=wq_ext[128 * ec : 128 * (ec + 1), :],
                )

            # Each HWDGE queue's DMAs serialize end-to-end (issue + DGE +
            # transfer + sem-prop ~2.3-2.9us per 256KB chunk), so delivery
            # order and LANE COUNT dominate. Three lanes: SP, ACT, and the
            # gpsimd SWDGE (Pool is otherwise idle in the prologue). Both
            # halves' Q(ec) consume wq(ec), so all 8 wq chunks load early.
            # junk tile for PE p-state warmup spins (contents irrelevant);
            # memset first so the Pool DMAs don't delay it
            junk = constp.tile([128, 256], bf16, tag="junk")
            nc.gpsimd.memset(junk[:], 0.0)
            # split chunk (0,0) at 512 so K's first matmul starts sooner
            nc.sync.dma_start(
                out=xt[:, 0, 0:512], in_=xt_ext[0:128, 0:512]
            )
            nc.scalar.dma_start(
                out=wk_sb[:], in_=wk_ext.rearrange("(c p) d -> p c d", p=128)
            )
            load_x(0, 1, nc.gpsimd)
            nc.sync.dma_start(
                out=xt[:, 0, 512:1024], in_=xt_ext[0:128, 512:1024]
            )
            load_wq(0)
            load_x(0, 3, nc.gpsimd)
            load_x(0, 2, nc.sync)
            load_wq(1)
            load_x(0, 5, nc.gpsimd)
            load_x(0, 4, nc.sync)
            load_wq(2)
            load_x(0, 7, nc.gpsimd)
            nc.sync.dma_start(
                out=wq_sb[:, 3, :], in_=wq_ext[128 * 3 : 128 * 4, :]
            )
            load_x(0, 6, nc.sync)
            load_wq(4)
            nc.sync.dma_start(out=rct_sb[:], in_=rct_ext[:])
            load_wq(5)
            nc.gpsimd.dma_start(
                out=wq_sb[:, 7, :], in_=wq_ext[128 * 7 : 128 * 8, :]
            )
            load_x(1, 0, nc.sync)
            load_wq(6)
            load_x(1, 5, nc.gpsimd)
            load_x(1, 7, nc.gpsimd)
            nc.scalar.dma_start(out=permt_sb[:], in_=permt_ext[:])
            nc.scalar.dma_start(
                out=wv_sb[:], in_=wv_ext.rearrange("(c p) d -> p c d", p=128)
            )
            load_x(1, 2, nc.sync)
            load_x(1, 1, nc.scalar)
            load_x(1, 4, nc.sync)
            load_x(1, 3, nc.scalar)
            load_x(1, 6, nc.sync)
            nc.sync.dma_start(out=rst_sb[:], in_=rst_ext[:])
            nc.scalar.dma_start(out=mask_sb[:], in_=mask_ext[:])
            nc.gpsimd.memset(vones[:, :, HD : HD + 1], 1.0)

            # ACT is busy with exp during attention; keep evacs on DVE.
            # (ACT is idle in the prologue, so V evacs go there via Copy.)
            def evac_dve(dst_ap, src_ap):
                nc.vector.tensor_copy(dst_ap, src_ap)

            def evac_act(dst_ap, src_ap):
                nc.scalar.activation(
                    dst_ap, src_ap, mybir.ActivationFunctionType.Copy
                )

            kt_un = unp.tile([128, L], bf16, tag="un")
            qun = unp.tile([128, L], bf16, tag="un", name="qun0")

            def rope_units(src_un, dst, rot_sb):
                # dst = src*Rc + (PermT.T @ src)*Rs, bf16 d-major, as
                # independently emittable units (pipeline fill work).
                # Chunked per 1024 cols so the consumer's first i-groups
                # unblock after chunk 0's add.
                def mul1(ch):
                    def f():
                        sl = slice(1024 * ch, 1024 * (ch + 1))
                        # Pool: dst = src * Rc (independent of the perm)
                        nc.gpsimd.tensor_tensor(
                            dst[:, sl], src_un[:, sl], rct_sb[:, sl], op=MUL
                        )
                    return f

                def permch(ch):
                    def f():
                        rp = pbig.tile([128, 1024], f32, tag="big", name="rp")
                        for q in range(2):
                            sl = slice(
                                1024 * ch + 512 * q, 1024 * ch + 512 * (q + 1)
                            )
                            nc.tensor.matmul(
                                rp[:, 512 * q : 512 * (q + 1)],
                                permt_sb[:],
                                src_un[:, sl],
                                start=True,
                                stop=True,
                            )
                        # fused evac: rot = psum * Rs (DVE)
                        nc.vector.tensor_tensor(
                            rot_sb[:, 1024 * ch : 1024 * (ch + 1)],
                            rp[:],
                            rst_sb[:, 1024 * ch : 1024 * (ch + 1)],
                            op=MUL,
                        )
                    return f

                def add(ch):
                    def f():
                        sl = slice(1024 * ch, 1024 * (ch + 1))
                        nc.vector.tensor_tensor(
                            dst[:, sl], dst[:, sl], rot_sb[:, sl], op=ADD
                        )
                    return f

                return [mul1(0), permch(0), mul1(1), permch(1), add(0), add(1)]

            # ---- prologue: K/Q0 projections interleaved per e-chunk ----
            # PSUM start_tensor_calc zeroes the whole 2KB bank, so only
            # groups that own a full bank may interleave: K's two 512-col
            # f32 groups (one bank each) and Q0's qA/qB (one bank each).
            # V's 8 j-blocks (4 per bank) must run block-outer, after all
            # chunks of the half have landed (which they have by then).
            def proj_half(h2):
                pk = pbig.tile([128, 1024], f32, tag="big", name=f"pk{h2}")
                qA = poutpA.tile([128, 512], f32, tag="poA", name=f"qA{h2}")
                qB = poutpB.tile([128, 512], f32, tag="poB", name=f"qB{h2}")
                w = 1024 * h2
                for ec in range(EC):
                    st = ec == 0
                    sp = ec == EC - 1
                    nc.tensor.matmul(
                        pk[:, 0:512], wk_sb[:, ec, :], xt[:, ec, w : w + 512],
                        start=st, stop=sp, skip_group_check=True,
                    )
                    nc.tensor.matmul(
                        pk[:, 512:1024], wk_sb[:, ec, :],
                        xt[:, ec, w + 512 : w + 1024],
                        start=st, stop=sp, skip_group_check=True,
                    )
                    nc.tensor.matmul(
                        qA[:, 0:512], wq_sb[:, ec, 0:128],
                        xt[:, ec, w : w + 512],
                        start=st, stop=sp, skip_group_check=True,
                    )
                    nc.tensor.matmul(
                        qB[:, 0:512], wq_sb[:, ec, 0:128],
                        xt[:, ec, w + 512 : w + 1024],
                        start=st, stop=sp, skip_group_check=True,
                    )
                evac_dve(kt_un[:, w : w + 1024], pk[:])
                evac_dve(qun[:, w : w + 512], qA[:])
                evac_dve(qun[:, w + 512 : w + 1024], qB[:])

            def v_half(h2):
                # V rows: stationary = xt chunk, 8 j-block groups per tile,
                # block-outer so each bank hosts strictly sequential groups
                pv = pbig.tile([128, 1024], f32, tag="big", name=f"pv{h2}")
                for m in range(8):
                    lb = 8 * h2 + m
                    for ec in range(EC):
                        nc.tensor.matmul(
                            pv[:, 128 * m : 128 * (m + 1)],
                            xt[:, ec, 128 * lb : 128 * (lb + 1)],
                            wv_sb[:, ec, :],
                            start=(ec == 0),
                            stop=(ec == EC - 1),
                            skip_group_check=True,
                        )
                evac_act(vones[:, 8 * h2 : 8 * h2 + 8, 0:HD], pv[:])

            # PE p-state warmup: spin on junk while the first loads land.
            # The real first matmul waits on wk/x anyway; these ramp the
            # tensor engine to full clock in the meantime.
            pjunk = pbig.tile([128, 1024], f32, tag="big", name="pjunk")
            for _ in range(28):
                nc.tensor.matmul(
                    pjunk[:, 0:128], junk[:, 0:128], junk[:, 128:256],
                    start=True, stop=True, skip_group_check=True,
                )

            krot = rotp.tile([128, L], bf16, tag="rot", name="krot")
            k_units = rope_units(kt_un, kt, krot)
            qt_cur = qtp.tile([128, L], bf16, tag="qt", name="qt0")
            qrot = rotp.tile([128, L], bf16, tag="rot", name="qrot0")
            q_units = rope_units(qun, qt_cur, qrot)
            # chunk-0 ropes for BOTH K and Q0 complete before half 1, so
            # head-0 attention groups 0-3 (cols 0:1024 only) are unblocked
            # by the time the PE reaches the first score matmul
            proj_half(0)
            k_units[0]()   # k mul1(0) on Pool
            k_units[1]()   # k permch(0): PE + DVE (fills the evac window)
            q_units[0]()   # q mul1(0)
            q_units[1]()   # q permch(0)
            k_units[4]()   # k add(0) DVE
            q_units[4]()   # q add(0) DVE
            v_half(0)
            proj_half(1)
            k_units[2]()   # k mul1(1)
            k_units[3]()   # k permch(1)
            q_units[2]()   # q mul1(1)
            q_units[3]()   # q permch(1)
            k_units[5]()   # k add(1)
            q_units[5]()   # q add(1)
            v_half(1)

            # ---- Q projection emission units (pipeline fill for heads 1+) --
            def q_proj_units(hl, qun_tile):
                units = []
                for ch in range(2):
                    def mk(ch=ch):
                        pk = pbig.tile(
                            [128, 1024], f32, tag="big", name=f"pq{hl}_{ch}"
                        )
                        for q in range(2):
                            w = 1024 * ch + 512 * q
                            for ec in range(EC):
                                nc.tensor.matmul(
                                    pk[:, 512 * q : 512 * (q + 1)],
                                    wq_sb[:, ec, 128 * hl : 128 * (hl + 1)],
                                    xt[:, ec, w : w + 512],
                                    start=(ec == 0),
                                    stop=(ec == EC - 1),
                                )
                        evac_dve(
                            qun_tile[:, 1024 * ch : 1024 * (ch + 1)], pk[:]
                        )
                    units.append(mk)
                return units

            # ---- attention per head with lookahead-3 pipeline ----
            LOOKAHEAD = 3

            def attention(hl, qt_t, fill_units, groups, act_groups=frozenset()):
                # tp list: (g, t0, nblocks, is_first, is_last)
                tps = []
                for g in groups:
                    n_t = 2 * g + 2
                    for t0 in range(0, n_t, 4):
                        nb = min(4, n_t - t0)
                        tps.append((g, t0, nb, t0 == 0, t0 + nb == n_t))
                n = len(tps)
                sc_tiles = [None] * n
                pt_tiles = [None] * n
                pout_tiles = {}
                fill = list(fill_units)
                # finish fills a few chunks before head end so the next
                # head's rope adds are ready at its first score matmul
                fill_start = max(0, n - 4 - len(fill))

                def emit_sc(i):
                    g, t0, nb, _, is_last = tps[i]
                    n_t = 2 * g + 2
                    sc = pbig.tile([128, 1024], f32, tag="big", name=f"sc{i}")
                    for s in range(nb):
                        t = t0 + s
                        if t == n_t - 1:
                            # odd diagonal block: half0 rows are fully
                            # masked; compute only the live 128 i-columns
                            nc.tensor.matmul(
                                sc[:, 256 * s : 256 * s + 128],
                                kt[:, 128 * t : 128 * (t + 1)],
                                qt_t[:, 256 * g + 128 : 256 * (g + 1)],
                                start=True,
                                stop=True,
                            )
                        else:
                            nc.tensor.matmul(
                                sc[:, 256 * s : 256 * (s + 1)],
                                kt[:, 128 * t : 128 * (t + 1)],
                                qt_t[:, 256 * g : 256 * (g + 1)],
                                start=True,
                                stop=True,
                            )
                    sc_tiles[i] = sc

                for i in range(-LOOKAHEAD, n):
                    j = i + LOOKAHEAD
                    if j < n:
                        emit_sc(j)
                    if i >= fill_start and fill:
                        fill.pop(0)()
                    if i < 0:
                        continue
                    g, t0, nb, is_first, is_last = tps[i]
                    w = 256 * nb - (128 if is_last else 0)
                    pt_t = ptp.tile([128, 1024], bf16, tag="pt")
                    nc.scalar.activation(
                        pt_t[:, 0:w], sc_tiles[i][:, 0:w], EXP, scale=SCALE
                    )
                    if is_last:
                        # mask the two diagonal j-blocks (last 384 used cols)
                        nc.vector.tensor_tensor(
                            pt_t[:, w - 384 : w],
                            pt_t[:, w - 384 : w],
                            mask_sb[:],
                            op=MUL,
                        )
                    pt_tiles[i] = pt_t
                    sc_tiles[i] = None
                    if is_first:
                        pout_tiles[g] = (
                            poutpA.tile(
                                [128, 512], f32, tag="poA", name=f"poA_{hl}_{g}"
                            ),
                            poutpB.tile(
                                [128, 512], f32, tag="poB", name=f"poB_{hl}_{g}"
                            ),
                        )
                    po = pout_tiles[g]
                    n_t = 2 * g + 2
                    for s in range(nb):
                        t = t0 + s
                        if t == n_t - 1:
                            # odd diagonal block: packed live half1 only
                            nc.tensor.matmul(
                                po[1][:, 0 : HD + 1],
                                pt_t[:, 256 * s : 256 * s + 128],
                                vones[:, t, :],
                                start=(t == 0),
                                stop=True,
                                skip_group_check=True,
                            )
                            continue
                        for half in range(2):
                            nc.tensor.matmul(
                                po[half][:, 0 : HD + 1],
                                pt_t[:, 256 * s + 128 * half : 256 * s + 128 * (half + 1)],
                                vones[:, t, :],
                                start=(t == 0),
                                stop=(t == n_t - 1 - (1 - half) and t != n_t - 1),
                                skip_group_check=True,
                            )
                    pt_tiles[i] = None
                    if is_last:
                        ob = ostp.tile([128, 2 * (HD + 1)], f32, tag="ob")
                        evac_dve(ob[:, 0 : HD + 1], po[0][:, 0 : HD + 1])
                        evac_dve(ob[:, HD + 1 : 2 * (HD + 1)], po[1][:, 0 : HD + 1])
                        for half in range(2):
                            row0 = L * hl + 256 * g + 128 * half
                            if half == 1 and g in act_groups:
                                # last store of the run goes to ACT: the
                                # Pool SWDGE drain is expensive (~3us) if
                                # its queue still has in-flight stores
                                eng = nc.scalar if g == 0 else nc.gpsimd
                            else:
                                eng = nc.sync
                            eng.dma_start(
                                out=out_ext[row0 : row0 + 128, :],
                                in_=ob[:, 129 * half : 129 * (half + 1)],
                            )
                        del pout_tiles[g]
                # leftover fill units (next head's remaining proj work)
                for u in fill:
                    u()

            if DEBUG:
                nc.sync.dma_start(out=dbg_kt_ext[:], in_=kt[:])
                nc.sync.dma_start(out=dbg_v_ext[:], in_=vones[:])
                nc.sync.dma_start(out=dbg_qt_ext[:], in_=qt_cur[:])
                nc.sync.dma_start(out=dbg_kun_ext[:], in_=kt_un[:])

            for hl in range(HPC):
                if hl + 1 < HPC:
                    qun_next = unp.tile(
                        [128, L], bf16, tag="un", name=f"qun{hl + 1}"
                    )
                    qt_next = qtp.tile(
                        [128, L], bf16, tag="qt", name=f"qt{hl + 1}"
                    )
                    qrot_next = rotp.tile(
                        [128, L], bf16, tag="rot", name=f"qrot{hl + 1}"
                    )
                    fill_units = q_proj_units(hl + 1, qun_next) + rope_units(
                        qun_next, qt_next, qrot_next
                    )
                    groups = range(NG)
                    act_groups = frozenset()
                else:
                    qt_next = None
                    fill_units = []
                    # last head: biggest groups first -> smallest tail; the
                    # bunched-up final small-group stores split SP/ACT
                    groups = range(NG - 1, -1, -1)
                    act_groups = frozenset({0, 1, 2, 3})
                attention(hl, qt_cur, fill_units, groups, act_groups)
                qt_cur = qt_next
    return nc


def _get_program():
    if "nc" not in _CACHE:
        _ensure_ntff_hook()
        _CACHE["nc"] = _build_program()
    return _CACHE["nc"]


def kernel(x, Wq, Wk, Wv, _trace=False):
    _ensure_ntff_hook()
    from concourse.bass_utils import run_bass_kernel_spmd

    nc = _get_program()
    rct, rst, permt, mask = _host_tables()
    xtb = [
        np.ascontiguousarray(np.asarray(x[b]).astype(BF16).T)  # (E, L)
        for b in range(B)
    ]
    wq_bf = np.asarray(Wq).astype(BF16)
    wk_bf = np.ascontiguousarray(np.asarray(Wk).astype(BF16))
    wv_bf = np.ascontiguousarray(np.asarray(Wv).astype(BF16))
    in_maps = []
    for c in range(N_CORES):
        b, hq = divmod(c, HPC)
        in_maps.append(
            {
                "xt": xtb[b],
                "wq": np.ascontiguousarray(
                    wq_bf[:, HPC * HD * hq : HPC * HD * (hq + 1)]
                ),
                "wk": wk_bf,
                "wv": wv_bf,
                "rct": rct,
                "rst": rst,
                "permt": permt,
                "mask": mask,
            }
        )
    res = run_bass_kernel_spmd(
        nc, in_maps, list(range(N_CORES)), trace=_trace
    )
    out = np.empty((B, L, NH * HD), np.float32)
    for c in range(N_CORES):
        b, hq = divmod(c, HPC)
        raw = res.results[c]["out"].reshape(HPC, L, HD + 1)
        vals = raw[:, :, :HD] / raw[:, :, HD : HD + 1]  # (4, L, 128)
        out[b, :, HPC * HD * hq : HPC * HD * (hq + 1)] = (
            vals.transpose(1, 0, 2).reshape(L, HPC * HD)
        )
    if _trace:
        return out, res
    return out


# revision 25
# speedup vs baseline: 1.0179x; 1.0179x over previous
"""Trainium2 Bass kernel for GroupRopeAttention (MQA + RoPE, causal).

Shapes (hardcoded): x (2, 2048, 1024), Wq (1024, 2048) -> 16 heads x 128,
Wk/Wv (1024, 128) single shared K/V head. Output (2, 2048, 2048).

Sharding: core c handles batch c//4 and query heads 4*(c%4)..4*(c%4)+3.
K/V are recomputed per core (no collectives). Each core returns a raw
(4*2048, 129) f32 slab = unnormalized PV output plus the softmax
denominator column; the host divides and reassembles.

Per-core pipeline (one TileContext, everything bf16 except PSUM):
  - x arrives HOST-pre-transposed (e-major xt in DRAM); per-e-chunk loads
    are split across the SP and ACT HWDGE queues in the order the PE
    consumes them, so the first projection matmul issues ~2us in
  - prologue: K^T / V / Q0^T projections interleaved per e-chunk (one
    moving pass per chunk each) accumulating in parallel PSUM regions;
    Q0 uses the attention pout banks which are idle during the prologue
  - RoPE (rotate-half = constant PermT matmul on PE; cos/sin tables are
    host-precomputed bf16 inputs) for K interleaved between the two
    i-halves of the prologue
  - per head: Q^T projection + RoPE as pipeline fill inside the previous
    head's attention; causal attention with a lookahead-3 software
    pipeline: scores (kt_t stationary, 256-col qt movings) -> exp on ACT
    (scale folded) -> causal mask via one DVE multiply with a constant
    [tril | ones | shifted-tril] bf16 tile -> PV (pt stationary, [V|1]
    moving) accumulating output + denominator in PSUM, DMA'd raw to DRAM
  - the last head walks its softmax groups largest-first so the kernel
    tail is the smallest group's exp/PV/store chain
"""

import sys
import types

sys.path.insert(0, "/opt/trn_rl_repo")

import numpy as np
import ml_dtypes

BF16 = ml_dtypes.bfloat16

B, L, E = 2, 2048, 1024
NH, HD = 16, 128
N_CORES = 8
HPC = 4          # heads per core
THETA = 10000.0
SCALE = 1.0 / float(np.sqrt(HD))
EC = E // 128    # 8 e-chunks
NJ = L // 128    # 16 j-blocks
NG = L // 256    # 8 i-groups per head

_CACHE = {}


def _ensure_ntff_hook():
    """Register the NTFF profile hook if the image's antenv lacks it."""
    try:
        from antenv.axon_hooks import get_axon_ntff_profile_hook  # noqa: F401
        return
    except ImportError:
        pass
    import antenv

    mod = types.ModuleType("antenv.axon_hooks")
    mod._hook = None

    def set_axon_ntff_profile_hook(h):
        mod._hook = h

    def get_axon_ntff_profile_hook():
        return mod._hook

    mod.set_axon_ntff_profile_hook = set_axon_ntff_profile_hook
    mod.get_axon_ntff_profile_hook = get_axon_ntff_profile_hook
    sys.modules["antenv.axon_hooks"] = mod
    antenv.axon_hooks = mod
    try:
        from trn_agent_boot.trn_boot import _ntff_profile_via_ctypes

        set_axon_ntff_profile_hook(
            _ntff_profile_via_ctypes("/opt/axon/libaxon_pjrt.so")
        )
    except Exception:
        pass


def _host_tables():
    freqs = 1.0 / THETA ** (np.arange(0, HD, 2, dtype=np.float64) / HD)  # (64,)
    t = np.arange(L, dtype=np.float64)
    f = t[:, None] * freqs[None, :]  # (L, 64)
    f = np.repeat(f, 2, axis=-1)  # (L, 128)
    rct = np.ascontiguousarray(np.cos(f).T.astype(BF16))  # (128, L)
    rst = np.ascontiguousarray(np.sin(f).T.astype(BF16))  # (128, L)
    # rot[d] = -src[d+1] for even d, +src[d-1] for odd d, via rot = PermT.T @ src
    permt = np.zeros((HD, HD), dtype=BF16)
    for k in range(HD // 2):
        permt[2 * k, 2 * k + 1] = BF16(1.0)
        permt[2 * k + 1, 2 * k] = BF16(-1.0)
    # causal mask window for the two diagonal j-blocks of an i-group:
    # cols 0:128 -> t==2g block, i-local 0..127: keep i >= j
    # cols 128:256 -> t==2g block, i-local 128..255: always kept
    # cols 256:512 -> t==2g+1 block, i-local 0..255: keep i-128 >= j
    j = np.arange(128)[:, None]
    m1 = (np.arange(128)[None, :] >= j)
    mask = np.concatenate(
        [m1, np.ones((128, 128), bool), m1], axis=1
    ).astype(BF16)  # (128, 384)
    return rct, rst, permt, mask


def _build_program():
    import concourse.bass as bass
    import concourse.mybir as mybir
    import concourse.tile as tile
    from concourse.vector_clock import ScopedClock

    MAX_DRAIN_WAITS = 1

    def _max_inst_waits(inst):
        return 1

    class PatchedTileContext(tile.TileContext):
        # This walrus build rejects >2 sync waits per instruction. After
        # scheduling, hoist excess waits onto preceding nops on the same
        # engine (engines execute in order, so semantics are identical).
        def schedule_and_allocate(self, validate_deps=False):
            ret = super().schedule_and_allocate(validate_deps=validate_deps)
            for blk in self.nc.m.functions[0].blocks:
                new_insts = []
                for inst in blk.instructions:
                    mw = _max_inst_waits(inst)
                    si = inst.sync_info
                    waits = list(si.on_wait) if si and si.on_wait else []
                    if len(waits) > mw:
                        n_extra = len(waits) - mw
                        for i in range(0, n_extra, mw):
                            nop = mybir.InstNoOp(
                                name=self.nc.get_next_instruction_name(),
                                ins=[],
                                outs=[],
                            )
                            nop.engine = inst.engine
                            nop.sync_info = mybir.SyncInfo(
                                on_wait=waits[i : min(i + mw, n_extra)],
                                on_update=[],
                            )
                            self.nc.register_instruction(nop, overwrite=True)
                            new_insts.append(nop)
                        inst.sync_info = mybir.SyncInfo(
                            on_wait=waits[n_extra:],
                            on_update=list(si.on_update or []),
                        )
                    new_insts.append(inst)
                blk.instructions = new_insts
            return ret

        # The tile-exit drain gets the same treatment but must stay last in
        # its engine stream, so split it during emission instead.
        def _drain_and_barrier(self, tick_clock, wait_clock):
            drain_inst = self.nc.sync.drain()
            wait_clock.add_sem_waits(
                drain_inst.ins, ScopedClock({None: tick_clock.global_clock})
            )
            si = drain_inst.ins.sync_info
            waits = list(si.on_wait) if si and si.on_wait else []
            if len(waits) > MAX_DRAIN_WAITS:
                drain_inst.ins.sync_info = mybir.SyncInfo(
                    on_wait=waits[:MAX_DRAIN_WAITS],
                    on_update=list(si.on_update or []),
                )
                for i in range(MAX_DRAIN_WAITS, len(waits), MAX_DRAIN_WAITS):
                    nop = self.nc.sync.nop()
                    nop.ins.sync_info = mybir.SyncInfo(
                        on_wait=waits[i : i + MAX_DRAIN_WAITS], on_update=[]
                    )
            self.nc.all_engine_barrier()
            assert self.sems is not None
            popped = self.nc._tile_sem_poison_stack.pop()
            assert popped is self._sem_poison
            self.nc.clear_and_free_semaphores(
                list(self.sems.allocated().values())
            )
            self.nc.all_engine_barrier()

    f32 = mybir.dt.float32
    bf16 = mybir.dt.bfloat16
    EXP = mybir.ActivationFunctionType.Exp
    MUL = mybir.AluOpType.mult
    ADD = mybir.AluOpType.add

    nc = bass.Bass("TRN2", num_devices=N_CORES)

    xt_ext = nc.declare_dram_parameter("xt", [E, L], bf16, isOutput=False)
    wq_ext = nc.declare_dram_parameter("wq", [E, HPC * HD], bf16, isOutput=False)
    wk_ext = nc.declare_dram_parameter("wk", [E, HD], bf16, isOutput=False)
    wv_ext = nc.declare_dram_parameter("wv", [E, HD], bf16, isOutput=False)
    rct_ext = nc.declare_dram_parameter("rct", [HD, L], bf16, isOutput=False)
    rst_ext = nc.declare_dram_parameter("rst", [HD, L], bf16, isOutput=False)
    permt_ext = nc.declare_dram_parameter("permt", [HD, HD], bf16, isOutput=False)
    mask_ext = nc.declare_dram_parameter("mask", [128, 384], bf16, isOutput=False)
    out_ext = nc.declare_dram_parameter("out", [HPC * L, HD + 1], f32, isOutput=True)
    import os
    DEBUG = bool(os.environ.get("KERNEL_DEBUG"))
    if DEBUG:
        dbg_kt_ext = nc.declare_dram_parameter("dbg_kt", [HD, L], bf16, isOutput=True)
        dbg_v_ext = nc.declare_dram_parameter("dbg_v", [128, NJ, HD + 1], bf16, isOutput=True)
        dbg_qt_ext = nc.declare_dram_parameter("dbg_qt", [HD, L], bf16, isOutput=True)
        dbg_kun_ext = nc.declare_dram_parameter("dbg_kun", [HD, L], bf16, isOutput=True)

    with PatchedTileContext(nc) as tc:
        with (
            tc.tile_pool(name="const", bufs=1) as constp,
            tc.tile_pool(name="un", bufs=2) as unp,
            tc.tile_pool(name="rot", bufs=2) as rotp,
            tc.tile_pool(name="qt", bufs=2) as qtp,
            tc.tile_pool(name="pt", bufs=6) as ptp,
            tc.tile_pool(name="ost", bufs=3) as ostp,
            tc.tile_pool(name="pbig", bufs=3, space="PSUM") as pbig,
            tc.tile_pool(name="poutA", bufs=1, space="PSUM") as poutpA,
            tc.tile_pool(name="poutB", bufs=1, space="PSUM") as poutpB,
        ):
            # ---- SBUF constants ----
            wk_sb = constp.tile([128, EC, HD], bf16, tag="wk")
            wv_sb = constp.tile([128, EC, HD], bf16, tag="wv")
            wq_sb = constp.tile([128, EC, HPC * HD], bf16, tag="wq")
            xt = constp.tile([128, EC, L], bf16, tag="xt")
            vones = constp.tile([128, NJ, HD + 1], bf16, tag="vones")
            kt = constp.tile([128, L], bf16, tag="kt")
            rct_sb = constp.tile([128, L], bf16, tag="rct")
            rst_sb = constp.tile([128, L], bf16, tag="rst")
            permt_sb = constp.tile([128, 128], bf16, tag="permt")
            mask_sb = constp.tile([128, 384], bf16, tag="mask")

            # ---- DMA loads, split across the SP and ACT HWDGE queues in
            # PE-consumption order (K/V/Q0 eat e-chunks of i-half 0 first) --
            def load_x(h2, ec, eng):
                eng.dma_start(
                    out=xt[:, ec, 1024 * h2 : 1024 * (h2 + 1)],
                    in_=xt_ext[
                        128 * ec : 128 * (ec + 1), 1024 * h2 : 1024 * (h2 + 1)
                    ],
                )

            def load_wq(ec):
                nc.scalar.dma_start(
                    out=wq_sb[:, ec, :],
                    in_=wq_ext[128 * ec : 128 * (ec + 1), :],
                )

            # Each HWDGE queue's DMAs serialize end-to-end (issue + DGE +
            # transfer + sem-prop ~2.3-2.9us per 256KB chunk), so delivery
            # order and LANE COUNT dominate. Three lanes: SP, ACT, and the
            # gpsimd SWDGE (Pool is otherwise idle in the prologue). Both
            # halves' Q(ec) consume wq(ec), so all 8 wq chunks load early.
            # junk tile for PE p-state warmup spins (contents irrelevant);
            # memset first so the Pool DMAs don't delay it
            junk = constp.tile([128, 256], bf16, tag="junk")
            nc.gpsimd.memset(junk[:], 0.0)
            # split chunk (0,0) at 512 so K's first matmul starts sooner
            nc.sync.dma_start(
                out=xt[:, 0, 0:512], in_=xt_ext[0:128, 0:512]
            )
            nc.scalar.dma_start(
                out=wk_sb[:], in_=wk_ext.rearrange("(c p) d -> p c d", p=128)
            )
            load_x(0, 1, nc.gpsimd)
            nc.sync.dma_start(
                out=xt[:, 0, 512:1024], in_=xt_ext[0:128, 512:1024]
            )
            load_wq(0)
            load_x(0, 3, nc.gpsimd)
            load_x(0, 2, nc.sync)
            load_wq(1)
            load_x(0, 5, nc.gpsimd)
            load_x(0, 4, nc.sync)
            load_wq(2)
            load_x(0, 7, nc.gpsimd)
            nc.sync.dma_start(
                out=wq_sb[:, 3, :], in_=wq_ext[128 * 3 : 128 * 4, :]
            )
            load_x(0, 6, nc.sync)
            load_wq(4)
            nc.sync.dma_start(out=rct_sb[:], in_=rct_ext[:])
            load_wq(5)
            nc.gpsimd.dma_start(
                out=wq_sb[:, 7, :], in_=wq_ext[128 * 7 : 128 * 8, :]
            )
            load_x(1, 0, nc.sync)
            load_wq(6)
            load_x(1, 5, nc.gpsimd)
            load_x(1, 7, nc.gpsimd)
            nc.scalar.dma_start(out=permt_sb[:], in_=permt_ext[:])
            nc.scalar.dma_start(
                out=wv_sb[:], in_=wv_ext.rearrange("(c p) d -> p c d", p=128)
            )
            load_x(1, 2, nc.sync)
            load_x(1, 1, nc.scalar)
            load_x(1, 4, nc.sync)
            load_x(1, 3, nc.scalar)
            load_x(1, 6, nc.sync)
            nc.sync.dma_start(out=rst_sb[:], in_=rst_ext[:])
            nc.scalar.dma_start(out=mask_sb[:], in_=mask_ext[:])
            nc.gpsimd.memset(vones[:, :, HD : HD + 1], 1.0)

            # ACT is busy with exp during attention; keep evacs on DVE.
            # (ACT is idle in the prologue, so V evacs go there via Copy.)
            def evac_dve(dst_ap, src_ap):
                nc.vector.tensor_copy(dst_ap, src_ap)

            def evac_act(dst_ap, src_ap):
                nc.scalar.activation(
                    dst_ap, src_ap, mybir.ActivationFunctionType.Copy
                )

            kt_un = unp.tile([128, L], bf16, tag="un")
            qun = unp.tile([128, L], bf16, tag="un", name="qun0")

            def rope_units(src_un, dst, rot_sb):
                # dst = src*Rc + (PermT.T @ src)*Rs, bf16 d-major, as
                # independently emittable units (pipeline fill work).
                # Chunked per 1024 cols so the consumer's first i-groups
                # unblock after chunk 0's add.
                def mul1(ch):
                    def f():
                        sl = slice(1024 * ch, 1024 * (ch + 1))
                        # Pool: dst = src * Rc (independent of the perm)
                        nc.gpsimd.tensor_tensor(
                            dst[:, sl], src_un[:, sl], rct_sb[:, sl], op=MUL
                        )
                    return f

                def permch(ch):
                    def f():
                        rp = pbig.tile([128, 1024], f32, tag="big", name="rp")
                        for q in range(2):
                            sl = slice(
                                1024 * ch + 512 * q, 1024 * ch + 512 * (q + 1)
                            )
                            nc.tensor.matmul(
                                rp[:, 512 * q : 512 * (q + 1)],
                                permt_sb[:],
                                src_un[:, sl],
                                start=True,
                                stop=True,
                            )
                        # fused evac: rot = psum * Rs (DVE)
                        nc.vector.tensor_tensor(
                            rot_sb[:, 1024 * ch : 1024 * (ch + 1)],
                            rp[:],
                            rst_sb[:, 1024 * ch : 1024 * (ch + 1)],
                            op=MUL,
                        )
                    return f

                def add(ch):
                    def f():
                        sl = slice(1024 * ch, 1024 * (ch + 1))
                        nc.vector.tensor_tensor(
                            dst[:, sl], dst[:, sl], rot_sb[:, sl], op=ADD
                        )
                    return f

                return [mul1(0), permch(0), mul1(1), permch(1), add(0), add(1)]

            # ---- prologue: K/Q0 projections interleaved per e-chunk ----
            # PSUM start_tensor_calc zeroes the whole 2KB bank, so only
            # groups that own a full bank may interleave: K's two 512-col
            # f32 groups (one bank each) and Q0's qA/qB (one bank each).
            # V's 8 j-blocks (4 per bank) must run block-outer, after all
            # chunks of the half have landed (which they have by then).
            def proj_half(h2):
                pk = pbig.tile([128, 1024], f32, tag="big", name=f"pk{h2}")
                qA = poutpA.tile([128, 512], f32, tag="poA", name=f"qA{h2}")
                qB = poutpB.tile([128, 512], f32, tag="poB", name=f"qB{h2}")
                w = 1024 * h2
                for ec in range(EC):
                    st = ec == 0
                    sp = ec == EC - 1
                    nc.tensor.matmul(
                        pk[:, 0:512], wk_sb[:, ec, :], xt[:, ec, w : w + 512],
                        start=st, stop=sp, skip_group_check=True,
                    )
                    nc.tensor.matmul(
                        pk[:, 512:1024], wk_sb[:, ec, :],
                        xt[:, ec, w + 512 : w + 1024],
                        start=st, stop=sp, skip_group_check=True,
                    )
                    nc.tensor.matmul(
                        qA[:, 0:512], wq_sb[:, ec, 0:128],
                        xt[:, ec, w : w + 512],
                        start=st, stop=sp, skip_group_check=True,
                    )
                    nc.tensor.matmul(
                        qB[:, 0:512], wq_sb[:, ec, 0:128],
                        xt[:, ec, w + 512 : w + 1024],
                        start=st, stop=sp, skip_group_check=True,
                    )
                evac_dve(kt_un[:, w : w + 1024], pk[:])
                evac_dve(qun[:, w : w + 512], qA[:])
                evac_dve(qun[:, w + 512 : w + 1024], qB[:])

            def v_half(h2):
                # V rows: stationary = xt chunk, 8 j-block groups per tile,
                # block-outer so each bank hosts strictly sequential groups
                pv = pbig.tile([128, 1024], f32, tag="big", name=f"pv{h2}")
                for m in range(8):
                    lb = 8 * h2 + m
                    for ec in range(EC):
                        nc.tensor.matmul(
                            pv[:, 128 * m : 128 * (m + 1)],
                            xt[:, ec, 128 * lb : 128 * (lb + 1)],
                            wv_sb[:, ec, :],
                            start=(ec == 0),
                            stop=(ec == EC - 1),
                            skip_group_check=True,
                        )
                evac_act(vones[:, 8 * h2 : 8 * h2 + 8, 0:HD], pv[:])

            # PE p-state warmup: spin on junk while the first loads land.
            # The real first matmul waits on wk/x anyway; these ramp the
            # tensor engine to full clock in the meantime.
            pjunk = pbig.tile([128, 1024], f32, tag="big", name="pjunk")
            for _ in range(28):
                nc.tensor.matmul(
                    pjunk[:, 0:128], junk[:, 0:128], junk[:, 128:256],
                    start=True, stop=True, skip_group_check=True,
                )

            krot = rotp.tile([128, L], bf16, tag="rot", name="krot")
            k_units = rope_units(kt_un, kt, krot)
            qt_cur = qtp.tile([128, L], bf16, tag="qt", name="qt0")
            qrot = rotp.tile([128, L], bf16, tag="rot", name="qrot0")
            q_units = rope_units(qun, qt_cur, qrot)
            # chunk-0 ropes for BOTH K and Q0 complete before half 1, so
            # head-0 attention groups 0-3 (cols 0:1024 only) are unblocked
            # by the time the PE reaches the first score matmul
            proj_half(0)
            k_units[0]()   # k mul1(0) on Pool
            k_units[1]()   # k permch(0): PE + DVE (fills the evac window)
            q_units[0]()   # q mul1(0)
            q_units[1]()   # q permch(0)
            k_units[4]()   # k add(0) DVE
            q_units[4]()   # q add(0) DVE
            v_half(0)
            proj_half(1)
            k_units[2]()   # k mul1(1)
            k_units[3]()   # k permch(1)
            q_units[2]()   # q mul1(1)
            q_units[3]()   # q permch(1)
            k_units[5]()   # k add(1)
            q_units[5]()   # q add(1)
            v_half(1)

            # ---- Q projection emission units (pipeline fill for heads 1+) --
            def q_proj_units(hl, qun_tile):
                units = []
                for ch in range(2):
                    def mk(ch=ch):
                        pk = pbig.tile(
                            [128, 1024], f32, tag="big", name=f"pq{hl}_{ch}"
                        )
                        for q in range(2):
                            w = 1024 * ch + 512 * q
                            for ec in range(EC):
                                nc.tensor.matmul(
                                    pk[:, 512 * q : 512 * (q + 1)],
                                    wq_sb[:, ec, 128 * hl : 128 * (hl + 1)],
                                    xt[:, ec, w : w + 512],
                                    start=(ec == 0),
                                    stop=(ec == EC - 1),
                                )
                        evac_dve(
                            qun_tile[:, 1024 * ch : 1024 * (ch + 1)], pk[:]
                        )
                    units.append(mk)
                return units

            # ---- attention per head with lookahead-3 pipeline ----
            LOOKAHEAD = 3

            def attention(hl, qt_t, fill_units, groups, act_groups=frozenset()):
                # tp list: (g, t0, nblocks, is_first, is_last)
                tps = []
                for g in groups:
                    n_t = 2 * g + 2
                    for t0 in range(0, n_t, 4):
                        nb = min(4, n_t - t0)
                        tps.append((g, t0, nb, t0 == 0, t0 + nb == n_t))
                n = len(tps)
                sc_tiles = [None] * n
                pt_tiles = [None] * n
                pout_tiles = {}
                fill = list(fill_units)
                # finish fills a few chunks before head end so the next
                # head's rope adds are ready at its first score matmul
                fill_start = max(0, n - 4 - len(fill))

                def emit_sc(i):
                    g, t0, nb, _, is_last = tps[i]
                    n_t = 2 * g + 2
                    sc = pbig.tile([128, 1024], f32, tag="big", name=f"sc{i}")
                    for s in range(nb):
                        t = t0 + s
                        if t == n_t - 1:
                            # odd diagonal block: half0 rows are fully
                            # masked; compute only the live 128 i-columns
                            nc.tensor.matmul(
                                sc[:, 256 * s : 256 * s + 128],
                                kt[:, 128 * t : 128 * (t + 1)],
                                qt_t[:, 256 * g + 128 : 256 * (g + 1)],
                                start=True,
                                stop=True,
                            )
                        else:
                            nc.tensor.matmul(
                                sc[:, 256 * s : 256 * (s + 1)],
                                kt[:, 128 * t : 128 * (t + 1)],
                                qt_t[:, 256 * g : 256 * (g + 1)],
                                start=True,
                                stop=True,
                            )
                    sc_tiles[i] = sc

                for i in range(-LOOKAHEAD, n):
                    j = i + LOOKAHEAD
                    if j < n:
                        emit_sc(j)
                    if i >= fill_start and fill:
                        fill.pop(0)()
                    if i < 0:
                        continue
                    g, t0, nb, is_first, is_last = tps[i]
                    w = 256 * nb - (128 if is_last else 0)
                    pt_t = ptp.tile([128, 1024], bf16, tag="pt")
                    nc.scalar.activation(
                        pt_t[:, 0:w], sc_tiles[i][:, 0:w], EXP, scale=SCALE
                    )
                    if is_last:
                        # mask the two diagonal j-blocks (last 384 used cols)
                        nc.vector.tensor_tensor(
                            pt_t[:, w - 384 : w],
                            pt_t[:, w - 384 : w],
                            mask_sb[:],
                            op=MUL,
                        )
                    pt_tiles[i] = pt_t
                    sc_tiles[i] = None
                    if is_first:
                        pout_tiles[g] = (
                            poutpA.tile(
                                [128, 512], f32, tag="poA", name=f"poA_{hl}_{g}"
                            ),
                            poutpB.tile(
                                [128, 512], f32, tag="poB", name=f"poB_{hl}_{g}"
                            ),
                        )
                    po = pout_tiles[g]
                    n_t = 2 * g + 2
                    for s in range(nb):
                        t = t0 + s
                        if t == n_t - 1:
                            # odd diagonal block: packed live half1 only
                            nc.tensor.matmul(
                                po[1][:, 0 : HD + 1],
                                pt_t[:, 256 * s : 256 * s + 128],
                                vones[:, t, :],
                                start=(t == 0),
                                stop=True,
                                skip_group_check=True,
                            )
                            continue
                        for half in range(2):
                            nc.tensor.matmul(
                                po[half][:, 0 : HD + 1],
                                pt_t[:, 256 * s + 128 * half : 256 * s + 128 * (half + 1)],
                                vones[:, t, :],
                                start=(t == 0),
                                stop=(t == n_t - 1 - (1 - half) and t != n_t - 1),
                                skip_group_check=True,
                            )
                    pt_tiles[i] = None
                    if is_last:
                        ob = ostp.tile([128, 2 * (HD + 1)], f32, tag="ob")
                        evac_dve(ob[:, 0 : HD + 1], po[0][:, 0 : HD + 1])
                        evac_dve(ob[:, HD + 1 : 2 * (HD + 1)], po[1][:, 0 : HD + 1])
                        for half in range(2):
                            row0 = L * hl + 256 * g + 128 * half
                            # tail stores ride the idle Pool SWDGE; ACT is
                            # exp-saturated in the last head and SP backs up
                            eng = (
                                nc.gpsimd
                                if (half == 1 and g in act_groups)
                                else nc.sync
                            )
                            eng.dma_start(
                                out=out_ext[row0 : row0 + 128, :],
                                in_=ob[:, 129 * half : 129 * (half + 1)],
                            )
                        del pout_tiles[g]
                # leftover fill units (next head's remaining proj work)
                for u in fill:
                    u()

            if DEBUG:
                nc.sync.dma_start(out=dbg_kt_ext[:], in_=kt[:])
                nc.sync.dma_start(out=dbg_v_ext[:], in_=vones[:])
                nc.sync.dma_start(out=dbg_qt_ext[:], in_=qt_cur[:])
                nc.sync.dma_start(out=dbg_kun_ext[:], in_=kt_un[:])

            for hl in range(HPC):
                if hl + 1 < HPC:
                    qun_next = unp.tile(
                        [128, L], bf16, tag="un", name=f"qun{hl + 1}"
                    )
                    qt_next = qtp.tile(
                        [128, L], bf16, tag="qt", name=f"qt{hl + 1}"
                    )
                    qrot_next = rotp.tile(
                        [128, L], bf16, tag="rot", name=f"qrot{hl + 1}"
                    )
                    fill_units = q_proj_units(hl + 1, qun_next) + rope_units(
                        qun_next, qt_next, qrot_next
                    )
                    groups = range(NG)
                    act_groups = frozenset()
                else:
                    qt_next = None
                    fill_units = []
                    # last head: biggest groups first -> smallest tail; the
                    # bunched-up final small-group stores split SP/ACT
                    groups = range(NG - 1, -1, -1)
                    act_groups = frozenset({0, 1, 2, 3})
                attention(hl, qt_cur, fill_units, groups, act_groups)
                qt_cur = qt_next
    return nc


def _get_program():
    if "nc" not in _CACHE:
        _ensure_ntff_hook()
        _CACHE["nc"] = _build_program()
    return _CACHE["nc"]


def kernel(x, Wq, Wk, Wv, _trace=False):
    _ensure_ntff_hook()
    from concourse.bass_utils import run_bass_kernel_spmd

    nc = _get_program()
    rct, rst, permt, mask = _host_tables()
    xtb = [
        np.ascontiguousarray(np.asarray(x[b]).astype(BF16).T)  # (E, L)
        for b in range(B)
    ]
    wq_bf = np.asarray(Wq).astype(BF16)
    wk_bf = np.ascontiguousarray(np.asarray(Wk).astype(BF16))
    wv_bf = np.ascontiguousarray(np.asarray(Wv).astype(BF16))
    in_maps = []
    for c in range(N_CORES):
        b, hq = divmod(c, HPC)
        in_maps.append(
            {
                "xt": xtb[b],
                "wq": np.ascontiguousarray(
                    wq_bf[:, HPC * HD * hq : HPC * HD * (hq + 1)]
                ),
                "wk": wk_bf,
                "wv": wv_bf,
                "rct": rct,
                "rst": rst,
                "permt": permt,
                "mask": mask,
            }
        )
    res = run_bass_kernel_spmd(
        nc, in_maps, list(range(N_CORES)), trace=_trace
    )
    out = np.empty((B, L, NH * HD), np.float32)
    for c in range(N_CORES):
        b, hq = divmod(c, HPC)
        raw = res.results[c]["out"].reshape(HPC, L, HD + 1)
        vals = raw[:, :, :HD] / raw[:, :, HD : HD + 1]  # (4, L, 128)
        out[b, :, HPC * HD * hq : HPC * HD * (hq + 1)] = (
            vals.transpose(1, 0, 2).reshape(L, HPC * HD)
        )
    if _trace:
        return out, res
    return out
